# revision 1
# baseline (speedup 1.0000x reference)
"""Trainium2 Bass kernel for nn_ConeIntersection.

Strategy: pure data-parallel over B (8 cores x 1024 tokens). Host pre-transposes
inputs to [N, DIM, B_local] (feature-major) so the device needs zero transposes;
the concat([axis - arg/2, axis + arg/2]) is folded into effective weights; the
mean-over-N of the gate branch is folded into PSUM accumulation across N.
All arithmetic fp32 (f32r matmuls run at full PE rate).
"""
import sys
sys.path.insert(0, '/opt/trn_rl_repo')
import numpy as np
from contextlib import ExitStack

N, B, DIM, HEADS = 4, 8192, 1024, 4
HD = DIM // HEADS            # 256
NCORES = 8
BL = B // NCORES             # 1024 tokens per core
TB = 256                     # token tile (matmul free dim)
NBT = BL // TB               # 4 mega-tiles
PI = 3.141592653589793
HALF_PI = PI / 2

_CACHE = {}


def _build():
    from concourse import bacc, tile, mybir
    f32 = mybir.dt.float32
    f32r = mybir.dt.float32r
    i32 = mybir.dt.int32
    AF = mybir.ActivationFunctionType
    ALU = mybir.AluOpType

    nc = bacc.Bacc("TRN2", target_bir_lowering=False, debug=False,
                   num_devices=NCORES)

    axisT_d = nc.dram_tensor("axisT", [N, DIM, BL], f32, kind="ExternalInput")
    argT_d = nc.dram_tensor("argT", [N, DIM, BL], f32, kind="ExternalInput")
    # effective weights, lhsT layout [in(K), out(M)] = [256, 256]
    wds = {}
    for wname in ["waax", "waar", "wgax", "wgar", "w2a", "w2g"]:
        wds[wname] = nc.dram_tensor(wname, [2 * 128, HD], f32, kind="ExternalInput")
    bds = {}
    for bname in ["b1a", "b1g", "b2a", "b2g"]:
        bds[bname] = nc.dram_tensor(bname, [2, 128], f32, kind="ExternalInput")
    axo_d = nc.dram_tensor("axis_outT", [DIM, BL], f32, kind="ExternalOutput")
    ago_d = nc.dram_tensor("arg_outT", [DIM, BL], f32, kind="ExternalOutput")

    with tile.TileContext(nc) as tc, ExitStack() as ctx:
        wpool = ctx.enter_context(tc.tile_pool(name="w", bufs=1))
        atp = ctx.enter_context(tc.tile_pool(name="atp", bufs=4))
        gtp = ctx.enter_context(tc.tile_pool(name="gtp", bufs=6))
        h1p = ctx.enter_context(tc.tile_pool(name="h1p", bufs=5))
        expp = ctx.enter_context(tc.tile_pool(name="expp", bufs=4))
        sump = ctx.enter_context(tc.tile_pool(name="sump", bufs=8))
        perp = ctx.enter_context(tc.tile_pool(name="perp", bufs=4))
        tmpp = ctx.enter_context(tc.tile_pool(name="tmpp", bufs=5))
        outp = ctx.enter_context(tc.tile_pool(name="outp", bufs=2))
        pmm = ctx.enter_context(tc.tile_pool(name="pmm", bufs=2, space="PSUM"))
        psc = ctx.enter_context(tc.tile_pool(name="psc", bufs=1, space="PSUM"))
        pgt = ctx.enter_context(tc.tile_pool(name="pgt", bufs=2, space="PSUM"))

        # resident weights / biases
        w_sb = {}
        for wname, wd in wds.items():
            tls = []
            for i in range(2):
                t = wpool.tile([128, HD], f32, tag=f"w_{wname}_{i}")
                nc.sync.dma_start(t[:].bitcast(f32r), wd[i * 128:(i + 1) * 128, :].bitcast(f32r))
                tls.append(t)
            w_sb[wname] = tls
        b_sb = {}
        for bname, bd in bds.items():
            tls = []
            for j in range(2):
                t = wpool.tile([128, 1], f32, tag=f"b_{bname}_{j}")
                nc.sync.dma_start(t[:], bd[j].unsqueeze(1))
                tls.append(t)
            b_sb[bname] = tls

        def mm_block(psum_ap, seq, start_k=True, stop_k=True):
            """seq: list of (w_tile, jblk, rhs_ap); accumulate into psum_ap."""
            nk = len(seq)
            for k, (wt, j, rhs) in enumerate(seq):
                nc.tensor.matmul(psum_ap, wt[:, j * 128:(j + 1) * 128].bitcast(f32r),
                                 rhs.bitcast(f32r),
                                 start=(start_k and k == 0), stop=(stop_k and k == nk - 1))

        for bt in range(NBT):
            t0 = bt * TB
            at_h = {}
            exp_h = {}
            minv = {}
            zg = {}
            Sc = {}
            Ss = {}
            ratio = {}
            corr = {}
            # ---------------- PHASE A (exp_and_others: relu, exp, copy, sign) ----
            for h in range(HEADS):
                f0 = h * HD
                at = atp.tile([128, N, 2, TB], f32, tag="at")
                gts = []
                for n in range(N):
                    src_a = axisT_d[n, f0:f0 + HD, t0:t0 + TB].rearrange(
                        "(j p) t -> p j t", p=128)
                    src_g = argT_d[n, f0:f0 + HD, t0:t0 + TB].rearrange(
                        "(j p) t -> p j t", p=128)
                    nc.sync.dma_start(at[:, n].bitcast(f32r), src_a.bitcast(f32r))
                    g = gtp.tile([128, 2, TB], f32, tag="gt")
                    nc.sync.dma_start(g[:].bitcast(f32r), src_g.bitcast(f32r))
                    gts.append(g)
                at_h[h] = at

                # min over n (DVE, runs alongside matmuls)
                mv = perp.tile([128, 2, TB], f32, tag="minv")
                nc.vector.tensor_tensor(mv[:], gts[0][:], gts[1][:], ALU.min)
                for n in range(2, N):
                    nc.vector.tensor_tensor(mv[:], mv[:], gts[n][:], ALU.min)
                minv[h] = mv

                # L1: h1a / h1g  [128(out j), N, TB]
                h1a, h1g = [], []
                for (wax, war, bias, hl) in (("waax", "waar", "b1a", h1a),
                                             ("wgax", "wgar", "b1g", h1g)):
                    for j in range(2):
                        pa = pmm.tile([128, N, TB], f32, tag="pmm")
                        # 4 weight blocks; one psum group per bank at a time:
                        # slices {0,1} share bank0, {2,3} bank1 -> pair (0,2), (1,3)
                        wseq = [(w_sb[wax][0], 0, "a"), (w_sb[wax][1], 1, "a"),
                                (w_sb[war][0], 0, "g"), (w_sb[war][1], 1, "g")]
                        for half in ((0, 2), (1, 3)):
                            for wb, (wt, i, which) in enumerate(wseq):
                                for n in half:
                                    rhs = at[:, n, i, :] if which == "a" else gts[n][:, i, :]
                                    nc.tensor.matmul(pa[:, n, :],
                                                     wt[:, j * 128:(j + 1) * 128].bitcast(f32r),
                                                     rhs.bitcast(f32r),
                                                     start=(wb == 0), stop=(wb == 3))
                        ht = h1p.tile([128, N, TB], f32, tag="h1")
                        nc.scalar.activation(ht[:].bitcast(f32r), pa[:], AF.Relu,
                                             bias=b_sb[bias][j][:])
                        hl.append(ht)

                # L2 scores -> exp ; gate accumulation
                ex = expp.tile([128, N, 2, TB], f32, tag="exp")
                for j in range(2):
                    ps = psc.tile([128, N, TB], f32, tag="psc")
                    for half in ((0, 2), (1, 3)):
                        for k, i in enumerate((0, 1)):
                            for n in half:
                                nc.tensor.matmul(ps[:, n, :],
                                                 w_sb["w2a"][i][:, j * 128:(j + 1) * 128].bitcast(f32r),
                                                 h1a[i][:, n, :].bitcast(f32r),
                                                 start=(k == 0), stop=(k == 1))
                    nc.scalar.activation(ex[:, :, j, :], ps[:], AF.Exp,
                                         bias=b_sb["b2a"][j][:])
                exp_h[h] = ex

                zgt = perp.tile([128, 2, TB], f32, tag="zg")
                for j in range(2):
                    pg = pgt.tile([128, TB], f32, tag="pgt")
                    k = 0
                    for i in range(2):
                        for n in range(N):
                            nc.tensor.matmul(pg[:],
                                             w_sb["w2g"][i][:, j * 128:(j + 1) * 128].bitcast(f32r),
                                             h1g[i][:, n, :].bitcast(f32r),
                                             start=(k == 0), stop=(k == 2 * N - 1))
                            k += 1
                    nc.scalar.activation(zgt[:, j, :], pg[:], AF.Identity,
                                         bias=b_sb["b2g"][j][:])
                zg[h] = zgt

            # ---------------- PHASE B (trig_and_small: sin, sign fillers) -------
            for h in range(HEADS):
                at = at_h[h]
                ex = exp_h[h]
                fl = lambda t: t[:].rearrange("p a b t -> p (a b t)")
                xw = tmpp.tile([128, N, 2, TB], f32, tag="bt")
                nc.vector.add_range_wrap(fl(xw), fl(at), 0.0, PI, 2 * PI)
                sinv = tmpp.tile([128, N, 2, TB], f32, tag="bt")
                nc.scalar.activation(fl(sinv), fl(xw), AF.Sin)
                xw2 = tmpp.tile([128, N, 2, TB], f32, tag="bt")
                nc.vector.add_range_wrap(fl(xw2), fl(at), HALF_PI, PI, 2 * PI)
                cosv = tmpp.tile([128, N, 2, TB], f32, tag="bt")
                nc.scalar.activation(fl(cosv), fl(xw2), AF.Sin)

                ec = tmpp.tile([128, N, 2, TB], f32, tag="bt")
                nc.vector.tensor_tensor(fl(ec), fl(ex), fl(cosv), ALU.mult)
                es = tmpp.tile([128, N, 2, TB], f32, tag="bt")
                nc.vector.tensor_tensor(fl(es), fl(ex), fl(sinv), ALU.mult)

                sc = perp.tile([128, 2, TB], f32, tag="Sc")
                ss = perp.tile([128, 2, TB], f32, tag="Ss")
                se = sump.tile([128, 2, TB], f32, tag="s2")
                nc.vector.tensor_tensor(sc[:], ec[:, 0], ec[:, 1], ALU.add)
                nc.vector.tensor_tensor(ss[:], es[:, 0], es[:, 1], ALU.add)
                nc.vector.tensor_tensor(se[:], ex[:, 0], ex[:, 1], ALU.add)
                for n in range(2, N):
                    nc.vector.tensor_tensor(sc[:], sc[:], ec[:, n], ALU.add)
                    nc.vector.tensor_tensor(ss[:], ss[:], es[:, n], ALU.add)
                    nc.vector.tensor_tensor(se[:], se[:], ex[:, n], ALU.add)
                Sc[h], Ss[h] = sc, ss

                # clamp: den = where(|Sc| < 0.001*Se, 0.001*Se, Sc)   (in-place on sc)
                nc.vector.tensor_scalar(se[:], se[:], 0.001, None, ALU.mult)  # th
                absc = sump.tile([128, 2, TB], f32, tag="s2")
                nc.vector.tensor_scalar(absc[:].bitcast(i32), sc[:].bitcast(i32),
                                        0x7FFFFFFF, None, ALU.bitwise_and)
                mask = sump.tile([128, 2, TB], i32, tag="s2")
                nc.vector.tensor_tensor(mask[:], absc[:], se[:], ALU.is_lt)
                nc.vector.copy_predicated(sc[:], mask[:], se[:])
                # octant-reduced atan2(Ss, den): |den| = max(|Sc|, th)
                ay = sump.tile([128, 2, TB], f32, tag="s2")
                nc.vector.tensor_scalar(ay[:].bitcast(i32), ss[:].bitcast(i32),
                                        0x7FFFFFFF, None, ALU.bitwise_and)
                ad = sump.tile([128, 2, TB], f32, tag="s2")
                nc.vector.tensor_tensor(ad[:], absc[:], se[:], ALU.max)
                mn = sump.tile([128, 2, TB], f32, tag="s2")
                nc.vector.tensor_tensor(mn[:], ay[:], ad[:], ALU.min)
                mx = sump.tile([128, 2, TB], f32, tag="s2")
                nc.vector.tensor_tensor(mx[:], ay[:], ad[:], ALU.max)
                nc.vector.reciprocal_approx_fast(mx[:], mx[:])
                u = sump.tile([128, 2, TB], f32, tag="s2")
                nc.vector.tensor_tensor(u[:], mn[:], mx[:], ALU.mult)
                a = outp.tile([128, 2, TB], f32, tag="ao")
                nc.scalar.activation(a[:], u[:], AF.Arctan)  # trig_and_small has arctan
                selm = sump.tile([128, 2, TB], i32, tag="s2")
                nc.vector.tensor_tensor(selm[:], ay[:], ad[:], ALU.is_gt)
                bb = sump.tile([128, 2, TB], f32, tag="s2")
                nc.vector.tensor_scalar(bb[:], a[:], -1.0, HALF_PI, ALU.mult, ALU.add)
                nc.vector.copy_predicated(a[:], selm[:], bb[:])   # theta' in [0, pi/2]
                pmt = sump.tile([128, 2, TB], f32, tag="s2")
                nc.vector.tensor_scalar(pmt[:], a[:], -1.0, PI, ALU.mult, ALU.add)
                indxm = sump.tile([128, 2, TB], i32, tag="s2")
                nc.vector.tensor_scalar(indxm[:], sc[:], 0.0, None, ALU.is_lt)
                nc.vector.copy_predicated(a[:], indxm[:], pmt[:])  # theta'' in [0, pi]
                sg = sump.tile([128, 2, TB], f32, tag="s2")
                nc.scalar.activation(sg[:], ss[:], AF.Sign)
                nc.vector.tensor_tensor(a[:], a[:], sg[:], ALU.mult)
                nc.sync.dma_start(
                    axo_d[h * HD:h * HD + HD, t0:t0 + TB].rearrange("(j p) t -> p j t", p=128),
                    a[:])

            # ---------------- PHASE C (sigmoid_and_others) ----------------------
            for h in range(HEADS):
                f0 = h * HD
                gg = outp.tile([128, 2, TB], f32, tag="gg")
                nc.scalar.activation(gg[:], zg[h][:], AF.Sigmoid)
                nc.vector.tensor_tensor(gg[:], gg[:], minv[h][:], ALU.mult)
                nc.sync.dma_start(
                    ago_d[f0:f0 + HD, t0:t0 + TB].rearrange("(j p) t -> p j t", p=128),
                    gg[:])

    nc.compile()
    return nc


def _get_nc():
    if "nc" not in _CACHE:
        _CACHE["nc"] = _build()
    return _CACHE["nc"]


def kernel(axis_embeddings, arg_embeddings, W_axis1, b_axis1, W_arg1, b_arg1,
           W_axis2, b_axis2, W_arg2, b_arg2, _return_results=False):
    from concourse.bass_utils import run_bass_kernel_spmd
    nc = _get_nc()

    f = np.float32
    W_axis1 = np.asarray(W_axis1, f); W_arg1 = np.asarray(W_arg1, f)
    W_axis2 = np.asarray(W_axis2, f); W_arg2 = np.asarray(W_arg2, f)
    # logits = [axis - arg/2, axis + arg/2]; fold concat into effective weights
    waax = np.ascontiguousarray((W_axis1[:, :HD] + W_axis1[:, HD:]).T)
    waar = np.ascontiguousarray(((W_axis1[:, HD:] - W_axis1[:, :HD]) / 2).T)
    wgax = np.ascontiguousarray((W_arg1[:, :HD] + W_arg1[:, HD:]).T)
    wgar = np.ascontiguousarray(((W_arg1[:, HD:] - W_arg1[:, :HD]) / 2).T)
    w2a = np.ascontiguousarray(W_axis2.T)
    w2g = np.ascontiguousarray((W_arg2 / N).T)     # folds mean over N
    weights = {"waax": waax, "waar": waar, "wgax": wgax, "wgar": wgar,
               "w2a": w2a, "w2g": w2g,
               "b1a": np.asarray(b_axis1, f).reshape(2, 128),
               "b1g": np.asarray(b_arg1, f).reshape(2, 128),
               "b2a": np.asarray(b_axis2, f).reshape(2, 128),
               "b2g": np.asarray(b_arg2, f).reshape(2, 128)}

    axis_embeddings = np.asarray(axis_embeddings, f)
    arg_embeddings = np.asarray(arg_embeddings, f)
    in_maps = []
    for c in range(NCORES):
        sl = slice(c * BL, (c + 1) * BL)
        m = dict(weights)
        m["axisT"] = np.ascontiguousarray(axis_embeddings[:, sl, :].transpose(0, 2, 1))
        m["argT"] = np.ascontiguousarray(arg_embeddings[:, sl, :].transpose(0, 2, 1))
        in_maps.append(m)

    res = run_bass_kernel_spmd(nc, in_maps, list(range(NCORES)))
    axis_out = np.empty((B, DIM), f)
    arg_out = np.empty((B, DIM), f)
    for c in range(NCORES):
        sl = slice(c * BL, (c + 1) * BL)
        axis_out[sl] = res.results[c]["axis_outT"].T
        arg_out[sl] = res.results[c]["arg_outT"].T
    if _return_results:
        return (axis_out, arg_out), res
    return axis_out, arg_out

